# revision 30
# baseline (speedup 1.0000x reference)
"""Trainium2 Bass kernel for CALayer (squeeze-excitation channel attention).

Reference computation (per batch sample b):
    y  = mean(x[b], spatial)              # [C]
    y1 = leaky_relu(w1 @ y + b1, 0.2)     # [16]
    y2 = sigmoid(w2 @ y1 + b2)            # [C]
    out[b] = x[b] * y2[:, None, None]

Sharding: data-parallel over batch — 8 samples, 8 NeuronCores, one sample per
core, weights replicated, no cross-core communication.

Per-core plan (memory-bound; per-core total DMA cap measured ~390-410 GB/s
regardless of direction mix — the 8 cores saturate the device HBM — so the
load and store phases are serial by nature and the only recoverable time
is overhead):
  - x[b] viewed as [256, 16384] lives in SBUF as two [128, 16384] channel
    halves, streamed in on the SP and ACT HWDGE rings (one half each,
    identical 2048-col chunk plans tapering to 256 so the last-arriving
    dependency is tiny).
  - Pooling: DVE reduce_sum per chunk, arrival-paced (its stream carries no
    DMA triggers, so it is immune to the ring-FIFO trigger blocking that
    stalls the ACT stream); ACT picks up only h1's last few chunks — their
    own triggers must issue before the data arrives, so ACT's sequencer is
    guaranteed past its trigger queue by then. Per-chunk partials land in
    part0/part1; one DVE row-reduce per half collapses them to ysum.
  - Gate: 2 fp32 PE matmuls (w1 halves, PSUM-accumulated) -> DVE bias+leaky
    (max(0.2t,t)) -> 2 bf16 PE matmuls vs w2 (b2 folded in via augmented
    ones row; bf16 halves the fp32-split LDWEIGHTS+MATMUL pairs, costing
    only ~4e-5 rel err) -> one bias-free Sigmoid on ACT (table pre-warmed
    at kernel start so no load sits in the gate).
  - Scale: per-partition broadcast multiply — DVE for half0, ACT for half1,
    in place, 256-col first chunk so stores ramp immediately after the
    gate; stores interleave on the two HWDGE rings, byte-balanced.

HBM traffic per core: 16 MiB in + 16 MiB out (the roofline for this op).
Fixed costs: ~5.6us NRT preamble + ~1.6us tile/table setup before the first
trigger, ~3us postamble — immovable from kernel content.
"""

from contextlib import ExitStack

import numpy as np

import concourse.bacc as bacc
import concourse.bass as bass
import concourse.mybir as mybir
import concourse.tile as tile
from concourse.bass_utils import run_bass_kernel_spmd

F32 = mybir.dt.float32
BF16 = mybir.dt.bfloat16
AF = mybir.ActivationFunctionType
ALU = mybir.AluOpType
AX = mybir.AxisListType

B, C, H, W = 8, 256, 128, 128
S = H * W          # 16384 spatial elements
CS = 16            # squeezed channels
NEG_SLOPE = 0.2
N_CORES = 8
P = 128            # SBUF partitions

# Load plan: widths per chunk, same for both halves (h0 on the SP/sync
# HWDGE ring, h1 on the ACT/scalar ring), tapering so the last-arriving
# dependency is tiny. HWDGE trigger instructions BLOCK their sequencer
# until the ring FIFO (~4 triggers / 512 descriptors) has room, and an
# engine cannot run work queued behind a blocked trigger — and the tile
# scheduler does not model that blocking, so it frontloads all triggers in
# each stream. Consequence: ACT (whose sequencer carries the scalar-ring
# triggers) cannot pool until its LAST load trigger unblocks (~75% through
# the load phase). So ALL pooling goes to DVE, whose stream has no
# triggers: ~37us of reduce work, arrival-paced against ~42us of loads,
# finishing ~0.8us after the last byte regardless of fleet congestion.
# ACT additionally pools h1's LAST four chunks: their own triggers must
# issue before the data can arrive, so ACT's sequencer is past its trigger
# queue by then — safe by construction, and it halves the serial pool tail.
# (The SWDGE ring is ~20x slower than a HWDGE ring — weights only.)
# h0/sync plan: uniform with a fine taper (SP sequencer has no compute, so
# its 11 blocking triggers are harmless; DVE pools these arrival-paced).
LOAD_H0 = [2048] * 7 + [1024, 512, 256, 256]
# h1/scalar plan: front-small / middle-big. The ACT sequencer's LAST
# trigger enters the ring FIFO when chunk n-5 completes — with the small
# chunks up front that is only ~37% into the ring (~24us), so ACT is free
# to pool everything from chunk 5 on (the two 4096s + tail, ~11.5us of
# work, arrival-paced with slack). DVE pools h0 plus h1's first five
# chunks and drops to ~64% utilization, leaving real slack to absorb
# congestion drift on either engine.
LOAD_H1 = [512, 1024, 1024, 1536, 2048, 4096, 4096, 1024, 512, 512]
ACT_POOL_FROM = 5      # h1 chunks >= this index pooled on ACT
# DVE pool emission order, interleaved by estimated DMA arrival time.
DVE_ORDER = [(1, 0), (1, 1), (1, 2), (0, 0), (1, 3), (0, 1), (1, 4),
             (0, 2), (0, 3), (0, 4), (0, 5), (0, 6), (0, 7), (0, 8),
             (0, 9), (0, 10)]

# Store plan widths (same for both halves): small first chunks so stores
# ramp right after the gate, small last chunks so neither ring ends on a
# straggler. Ring/engine swaps keep both rings at 16384 columns total.
STORE_W = [256, 1280, 2048, 2048, 2048, 2048, 2048, 2048, 1024, 1024, 512]
SWAP_IDX = 5       # mid chunk swapped to the other half's ring
LAST = len(STORE_W) - 1


def _offs(widths):
    offs, o = [], 0
    for w in widths:
        offs.append(o)
        o += w
    assert o == S
    return offs


def _body(tc, x, w1t, b1, w2b, out):
    """Emit the per-core kernel. APs: x/out [C, S], w1t [C, CS], b1 [CS, 1],
    w2b [CS+1, C] (w2 transposed with b2 appended as the last row)."""
    nc = tc.nc
    xr = x.rearrange("(h p) s -> h p s", p=P)       # [2, 128, S]
    outr = out.rearrange("(h p) s -> h p s", p=P)

    with ExitStack() as ctx:
        data = ctx.enter_context(tc.tile_pool(name="data", bufs=1))
        small = ctx.enter_context(tc.tile_pool(name="small", bufs=1))
        psum = ctx.enter_context(tc.tile_pool(name="psum", bufs=1, space="PSUM"))

        # Persistent SBUF halves of x (channel c on partition, spatial on free)
        xt0 = data.tile([P, S], F32)
        xt1 = data.tile([P, S], F32)

        # Constants. w1t packed [p, h, CS] so one DMA loads both halves.
        # w2 is augmented with b2 as a 17th contraction row (rhs gets a
        # constant 1.0) so mm2 computes w2@y1 + b2 directly and the two
        # sigmoids collapse into one bias-free ACTIVATE.
        w1_raw = small.tile([P, 2, CS], F32)
        w2b_raw = small.tile([CS + 1, C], F32)
        w1_sb = small.tile([P, 2, CS], F32)
        # mm2 runs in bf16: halves the fp32-split LDWEIGHTS+MATMUL pairs on
        # the gate critical path. Numerically safe — y1/w2 are O(0.1), so
        # the bf16 rounding perturbs sigmoid's output by ~1e-4 relative.
        w2b_sb = small.tile([CS + 1, C], BF16)
        b1_sb = small.tile([CS, 1], F32)
        nc.gpsimd.dma_start(out=w1_raw, in_=w1t.rearrange("(h p) c -> p h c", p=P))
        nc.gpsimd.dma_start(out=w2b_raw, in_=w2b)
        nc.gpsimd.dma_start(out=b1_sb, in_=b1)
        # Stage the matmul weights through DVE: PE LDWEIGHTS can encode only
        # ONE sync wait, so every matmul must depend on a single semaphore
        # (DVE's) — never on a DMA-lane sem + DVE at once.
        nc.vector.tensor_copy(w1_sb, w1_raw)
        nc.vector.tensor_copy(w2b_sb, w2b_raw)

        # Warm the ACT sigmoid table set BEFORE the scalar-ring triggers so
        # the table load runs during the preamble, not after the triggers
        # unblock (must be ACT's first ACTIVATE so Sigmoid/Copy share one
        # table-set load).
        warm = small.tile([1, 1], F32)
        nc.vector.memset(warm, 0.0)
        nc.scalar.activation(out=warm, in_=warm, func=AF.Sigmoid)

        # Phase A: load x + pool.
        offs0, offs1 = _offs(LOAD_H0), _offs(LOAD_H1)
        n0, n1 = len(LOAD_H0), len(LOAD_H1)
        part0 = small.tile([P, n0], F32)
        part1 = small.tile([P, n1], F32)

        for j in range(max(n0, n1)):
            if j < n0:
                sl = slice(offs0[j], offs0[j] + LOAD_H0[j])
                nc.sync.dma_start(out=xt0[:, sl], in_=xr[0, :, sl])
            if j < n1:
                sl = slice(offs1[j], offs1[j] + LOAD_H1[j])
                nc.scalar.dma_start(out=xt1[:, sl], in_=xr[1, :, sl])

        # DVE pools arrival-paced (read-only reduce: in-place DVE
        # tensor_scalar with accum_out is 2x faster but its SBUF write
        # traffic throttles the concurrent load DMAs). ACT takes h1's
        # middle/tail chunks via Copy-with-accum_out to a rotating scratch.
        scr_pool = ctx.enter_context(tc.tile_pool(name="scratch", bufs=2))
        for h, j in DVE_ORDER:
            xt, offs, plan, part = (
                (xt0, offs0, LOAD_H0, part0) if h == 0
                else (xt1, offs1, LOAD_H1, part1)
            )
            sl = slice(offs[j], offs[j] + plan[j])
            nc.vector.reduce_sum(
                out=part[:, j : j + 1], in_=xt[:, sl], axis=AX.X
            )
        for j in range(ACT_POOL_FROM, n1):
            w = LOAD_H1[j]
            sl = slice(offs1[j], offs1[j] + w)
            scr = scr_pool.tile([P, 4096], F32, tag="scr")
            nc.scalar.activation(
                out=scr[:, :w], in_=xt1[:, sl], func=AF.Copy,
                bias=0.0, scale=1.0, accum_out=part1[:, j : j + 1],
            )

        # Gate. Collapse the partials to per-half sums with two DVE
        # row-reduces (they run right after the last 256-col pool), then
        # py1 = w1h0^T @ ysum0 + w1h1^T @ ysum1 as 2 PSUM-accumulated
        # matmuls. Each matmul waits only on DVE per the LDWEIGHTS rule.
        ysum = small.tile([P, 2], F32)
        nc.vector.reduce_sum(out=ysum[:, 0:1], in_=part0, axis=AX.X)
        nc.vector.reduce_sum(out=ysum[:, 1:2], in_=part1, axis=AX.X)

        # y1e is y1 with a constant 1.0 appended (row CS) to pick up the b2
        # row of the augmented w2b in mm2 (bf16 to match w2b_sb).
        y1e = small.tile([CS + 1, 1], BF16)
        nc.vector.memset(y1e, 1.0)  # row CS stays 1.0; rows :CS overwritten

        py1 = psum.tile([CS, 1], F32)
        nc.tensor.matmul(py1, w1_sb[:, 0, :], ysum[:, 0:1], start=True, stop=False)
        nc.tensor.matmul(py1, w1_sb[:, 1, :], ysum[:, 1:2], start=False, stop=True)

        # t = py1/s + b1 ; y1 = max(0.2*t, t)  (== leaky_relu(t))
        t = small.tile([CS, 1], F32)
        nc.vector.tensor_scalar(t, py1, 1.0 / S, b1_sb, ALU.mult, ALU.add)
        nc.vector.scalar_tensor_tensor(
            out=y1e[:CS, :], in0=t, scalar=NEG_SLOPE, in1=t,
            op0=ALU.mult, op1=ALU.max,
        )

        py2 = psum.tile([P, 2], F32)
        nc.tensor.matmul(py2[:, 0:1], w2b_sb[:, 0:P], y1e, start=True, stop=True)
        nc.tensor.matmul(py2[:, 1:2], w2b_sb[:, P : 2 * P], y1e, start=True, stop=True)

        y2_sb = small.tile([P, 2], F32)
        nc.scalar.activation(out=y2_sb, in_=py2, func=AF.Sigmoid)

        # Phase B: scale x by the gate in place and store, chunked so DMA-out
        # overlaps the multiplies. DVE scales half0 (+ half1's last chunk so
        # the phase never ends on ACT's serial chain); ACT scales the rest of
        # half1. Stores interleave on the two HWDGE rings with one mid-chunk
        # and the last chunk swapped across rings for byte balance.
        soffs = _offs(STORE_W)
        for c, w in enumerate(STORE_W):
            sl = slice(soffs[c], soffs[c] + w)
            nc.vector.tensor_scalar_mul(
                out=xt0[:, sl], in0=xt0[:, sl], scalar1=y2_sb[:, 0:1]
            )
            if c == LAST:
                nc.vector.tensor_scalar_mul(
                    out=xt1[:, sl], in0=xt1[:, sl], scalar1=y2_sb[:, 1:2]
                )
            else:
                nc.scalar.activation(
                    out=xt1[:, sl], in_=xt1[:, sl], func=AF.Copy, bias=0.0,
                    scale=y2_sb[:, 1:2],
                )
            r0, r1 = (nc.sync, nc.scalar)
            if c in (SWAP_IDX, LAST):
                r0, r1 = r1, r0
            r0.dma_start(out=outr[0, :, sl], in_=xt0[:, sl])
            r1.dma_start(out=outr[1, :, sl], in_=xt1[:, sl])


def build_calayer_bass(trn_type="TRN2"):
    # Bacc (not raw Bass): its compile() pipeline splits multi-wait sync_info
    # into event semaphores — TRN2 instructions encode at most one wait.
    nc = bacc.Bacc(trn_type=trn_type)
    x = nc.dram_tensor("x", [C, S], F32, kind="ExternalInput")
    w1t = nc.dram_tensor("w1t", [C, CS], F32, kind="ExternalInput")
    b1 = nc.dram_tensor("b1", [CS, 1], F32, kind="ExternalInput")
    w2b = nc.dram_tensor("w2b", [CS + 1, C], F32, kind="ExternalInput")
    out = nc.dram_tensor("out", [C, S], F32, kind="ExternalOutput")
    with tile.TileContext(nc) as tc:
        _body(tc, x[:, :], w1t[:, :], b1[:, :], w2b[:, :], out[:, :])
    nc.finalize()  # Bacc.finalize runs compile(): wait-splitting, reg alloc
    return nc


_NC_CACHE = None
RUN_KWARGS = {}      # test harness may inject trace=True etc.
LAST_RESULT = None   # BassKernelResults of the most recent run


def _get_nc():
    global _NC_CACHE
    if _NC_CACHE is None:
        _NC_CACHE = build_calayer_bass()
    return _NC_CACHE


def kernel(x, w1, b1, w2, b2):
    global LAST_RESULT
    x = np.asarray(x, dtype=np.float32)
    xf = np.ascontiguousarray(x.reshape(B, C, S))
    w1t_h = np.ascontiguousarray(np.asarray(w1, dtype=np.float32).T)  # [C, CS]
    w2t_h = np.asarray(w2, dtype=np.float32).T  # [CS, C]
    b1_h = np.ascontiguousarray(np.asarray(b1, dtype=np.float32).reshape(CS, 1))
    b2r = np.asarray(b2, dtype=np.float32).reshape(1, C)
    w2b_h = np.ascontiguousarray(np.concatenate([w2t_h, b2r], axis=0))  # [CS+1, C]

    in_maps = [
        {"x": xf[b], "w1t": w1t_h, "b1": b1_h, "w2b": w2b_h}
        for b in range(B)
    ]
    res = run_bass_kernel_spmd(
        _get_nc(), in_maps, core_ids=list(range(N_CORES)), **RUN_KWARGS
    )
    LAST_RESULT = res
    out = np.stack([res.results[b]["out"] for b in range(B)], axis=0)
    return out.reshape(B, C, H, W)


# revision 32
# speedup vs baseline: 1.1012x; 1.1012x over previous
"""Trainium2 Bass kernel for CALayer (squeeze-excitation channel attention).

Reference computation (per batch sample b):
    y  = mean(x[b], spatial)              # [C]
    y1 = leaky_relu(w1 @ y + b1, 0.2)     # [16]
    y2 = sigmoid(w2 @ y1 + b2)            # [C]
    out[b] = x[b] * y2[:, None, None]

Sharding: data-parallel over batch — 8 samples, 8 NeuronCores, one sample per
core, weights replicated, no cross-core communication.

Per-core plan (memory-bound; per-core total DMA cap measured ~390-410 GB/s
regardless of direction mix — the 8 cores saturate the device HBM — so the
load and store phases are serial by nature and the only recoverable time
is overhead):
  - x[b] viewed as [256, 16384] lives in SBUF as two [128, 16384] channel
    halves, streamed in on the SP and ACT HWDGE rings (one half each,
    identical 2048-col chunk plans tapering to 256 so the last-arriving
    dependency is tiny).
  - Pooling: DVE reduce_sum per chunk, arrival-paced (its stream carries no
    DMA triggers, so it is immune to the ring-FIFO trigger blocking that
    stalls the ACT stream); ACT picks up only h1's last few chunks — their
    own triggers must issue before the data arrives, so ACT's sequencer is
    guaranteed past its trigger queue by then. Per-chunk partials land in
    part0/part1; one DVE row-reduce per half collapses them to ysum.
  - Gate: 2 fp32 PE matmuls (w1 halves, PSUM-accumulated) -> DVE bias+leaky
    (max(0.2t,t)) -> 2 bf16 PE matmuls vs w2 (b2 folded in via augmented
    ones row; bf16 halves the fp32-split LDWEIGHTS+MATMUL pairs, costing
    only ~4e-5 rel err) -> one bias-free Sigmoid on ACT (table pre-warmed
    at kernel start so no load sits in the gate).
  - Scale: per-partition broadcast multiply — DVE for half0, ACT for half1,
    in place, 256-col first chunk so stores ramp immediately after the
    gate; stores interleave on the two HWDGE rings, byte-balanced.

HBM traffic per core: 16 MiB in + 16 MiB out (the roofline for this op).
Fixed costs: ~5.6us NRT preamble + ~1.6us tile/table setup before the first
trigger, ~3us postamble — immovable from kernel content.
"""

from contextlib import ExitStack

import numpy as np

import concourse.bacc as bacc
import concourse.bass as bass
import concourse.mybir as mybir
import concourse.tile as tile
from concourse.bass_utils import run_bass_kernel_spmd

F32 = mybir.dt.float32
BF16 = mybir.dt.bfloat16
AF = mybir.ActivationFunctionType
ALU = mybir.AluOpType
AX = mybir.AxisListType

B, C, H, W = 8, 256, 128, 128
S = H * W          # 16384 spatial elements
CS = 16            # squeezed channels
NEG_SLOPE = 0.2
N_CORES = 8
P = 128            # SBUF partitions

# Load plan: widths per chunk, same for both halves (h0 on the SP/sync
# HWDGE ring, h1 on the ACT/scalar ring), tapering so the last-arriving
# dependency is tiny. HWDGE trigger instructions BLOCK their sequencer
# until the ring FIFO (~4 triggers / 512 descriptors) has room, and an
# engine cannot run work queued behind a blocked trigger — and the tile
# scheduler does not model that blocking, so it frontloads all triggers in
# each stream. Consequence: ACT (whose sequencer carries the scalar-ring
# triggers) cannot pool until its LAST load trigger unblocks (~75% through
# the load phase). So ALL pooling goes to DVE, whose stream has no
# triggers: ~37us of reduce work, arrival-paced against ~42us of loads,
# finishing ~0.8us after the last byte regardless of fleet congestion.
# ACT additionally pools h1's LAST four chunks: their own triggers must
# issue before the data can arrive, so ACT's sequencer is past its trigger
# queue by then — safe by construction, and it halves the serial pool tail.
# (The SWDGE ring is ~20x slower than a HWDGE ring — weights only.)
LOAD_W = [2048] * 7 + [1024, 512, 256, 256]
ACT_POOL_TAIL = 4      # last N h1 chunks pooled on ACT instead of DVE

# Store plan widths (same for both halves): small first chunks so stores
# ramp right after the gate, small last chunks so neither ring ends on a
# straggler. Ring/engine swaps keep both rings at 16384 columns total.
STORE_W = [256, 1280, 2048, 2048, 2048, 2048, 2048, 2048, 1024, 1024, 512]
SWAP_IDX = 5       # mid chunk swapped to the other half's ring
LAST = len(STORE_W) - 1


def _offs(widths):
    offs, o = [], 0
    for w in widths:
        offs.append(o)
        o += w
    assert o == S
    return offs


def _body(tc, x, w1t, b1, w2b, out):
    """Emit the per-core kernel. APs: x/out [C, S], w1t [C, CS], b1 [CS, 1],
    w2b [CS+1, C] (w2 transposed with b2 appended as the last row)."""
    nc = tc.nc
    xr = x.rearrange("(h p) s -> h p s", p=P)       # [2, 128, S]
    outr = out.rearrange("(h p) s -> h p s", p=P)

    with ExitStack() as ctx:
        data = ctx.enter_context(tc.tile_pool(name="data", bufs=1))
        small = ctx.enter_context(tc.tile_pool(name="small", bufs=1))
        psum = ctx.enter_context(tc.tile_pool(name="psum", bufs=1, space="PSUM"))

        # Persistent SBUF halves of x (channel c on partition, spatial on free)
        xt0 = data.tile([P, S], F32)
        xt1 = data.tile([P, S], F32)

        # Constants. w1t packed [p, h, CS] so one DMA loads both halves.
        # w2 is augmented with b2 as a 17th contraction row (rhs gets a
        # constant 1.0) so mm2 computes w2@y1 + b2 directly and the two
        # sigmoids collapse into one bias-free ACTIVATE.
        w1_raw = small.tile([P, 2, CS], F32)
        w2b_raw = small.tile([CS + 1, C], F32)
        w1_sb = small.tile([P, 2, CS], F32)
        # mm2 runs in bf16: halves the fp32-split LDWEIGHTS+MATMUL pairs on
        # the gate critical path. Numerically safe — y1/w2 are O(0.1), so
        # the bf16 rounding perturbs sigmoid's output by ~1e-4 relative.
        w2b_sb = small.tile([CS + 1, C], BF16)
        b1_sb = small.tile([CS, 1], F32)
        nc.gpsimd.dma_start(out=w1_raw, in_=w1t.rearrange("(h p) c -> p h c", p=P))
        nc.gpsimd.dma_start(out=w2b_raw, in_=w2b)
        nc.gpsimd.dma_start(out=b1_sb, in_=b1)
        # Stage the matmul weights through DVE: PE LDWEIGHTS can encode only
        # ONE sync wait, so every matmul must depend on a single semaphore
        # (DVE's) — never on a DMA-lane sem + DVE at once.
        nc.vector.tensor_copy(w1_sb, w1_raw)
        nc.vector.tensor_copy(w2b_sb, w2b_raw)

        # Warm the ACT sigmoid table set BEFORE the scalar-ring triggers so
        # the table load runs during the preamble, not after the triggers
        # unblock (must be ACT's first ACTIVATE so Sigmoid/Copy share one
        # table-set load).
        warm = small.tile([1, 1], F32)
        nc.vector.memset(warm, 0.0)
        nc.scalar.activation(out=warm, in_=warm, func=AF.Sigmoid)

        # Phase A: load x + pool.
        offs = _offs(LOAD_W)
        nw = len(LOAD_W)
        part0 = small.tile([P, nw], F32)
        part1 = small.tile([P, nw], F32)

        for j, w in enumerate(LOAD_W):
            sl = slice(offs[j], offs[j] + w)
            nc.sync.dma_start(out=xt0[:, sl], in_=xr[0, :, sl])
            nc.scalar.dma_start(out=xt1[:, sl], in_=xr[1, :, sl])

        # DVE pools arrival-paced (read-only reduce: in-place DVE
        # tensor_scalar with accum_out is 2x faster but its SBUF write
        # traffic throttles the concurrent load DMAs). ACT takes h1's tail
        # chunks via Copy-with-accum_out to a rotating scratch.
        scr_pool = ctx.enter_context(tc.tile_pool(name="scratch", bufs=2))
        for j, w in enumerate(LOAD_W):
            sl = slice(offs[j], offs[j] + w)
            nc.vector.reduce_sum(
                out=part0[:, j : j + 1], in_=xt0[:, sl], axis=AX.X
            )
            if j < nw - ACT_POOL_TAIL:
                nc.vector.reduce_sum(
                    out=part1[:, j : j + 1], in_=xt1[:, sl], axis=AX.X
                )
        for j in range(nw - ACT_POOL_TAIL, nw):
            w = LOAD_W[j]
            sl = slice(offs[j], offs[j] + w)
            scr = scr_pool.tile([P, 1024], F32, tag="scr")
            nc.scalar.activation(
                out=scr[:, :w], in_=xt1[:, sl], func=AF.Copy,
                bias=0.0, scale=1.0, accum_out=part1[:, j : j + 1],
            )

        # Gate. Collapse the partials to per-half sums with two DVE
        # row-reduces (they run right after the last 256-col pool), then
        # py1 = w1h0^T @ ysum0 + w1h1^T @ ysum1 as 2 PSUM-accumulated
        # matmuls. Each matmul waits only on DVE per the LDWEIGHTS rule.
        ysum = small.tile([P, 2], F32)
        nc.vector.reduce_sum(out=ysum[:, 0:1], in_=part0, axis=AX.X)
        nc.vector.reduce_sum(out=ysum[:, 1:2], in_=part1, axis=AX.X)

        # y1e is y1 with a constant 1.0 appended (row CS) to pick up the b2
        # row of the augmented w2b in mm2 (bf16 to match w2b_sb).
        y1e = small.tile([CS + 1, 1], BF16)
        nc.vector.memset(y1e, 1.0)  # row CS stays 1.0; rows :CS overwritten

        py1 = psum.tile([CS, 1], F32)
        nc.tensor.matmul(py1, w1_sb[:, 0, :], ysum[:, 0:1], start=True, stop=False)
        nc.tensor.matmul(py1, w1_sb[:, 1, :], ysum[:, 1:2], start=False, stop=True)

        # t = py1/s + b1 ; y1 = max(0.2*t, t)  (== leaky_relu(t))
        t = small.tile([CS, 1], F32)
        nc.vector.tensor_scalar(t, py1, 1.0 / S, b1_sb, ALU.mult, ALU.add)
        nc.vector.scalar_tensor_tensor(
            out=y1e[:CS, :], in0=t, scalar=NEG_SLOPE, in1=t,
            op0=ALU.mult, op1=ALU.max,
        )

        py2 = psum.tile([P, 2], F32)
        nc.tensor.matmul(py2[:, 0:1], w2b_sb[:, 0:P], y1e, start=True, stop=True)
        nc.tensor.matmul(py2[:, 1:2], w2b_sb[:, P : 2 * P], y1e, start=True, stop=True)

        y2_sb = small.tile([P, 2], F32)
        nc.scalar.activation(out=y2_sb, in_=py2, func=AF.Sigmoid)

        # Phase B: scale x by the gate in place and store, chunked so DMA-out
        # overlaps the multiplies. DVE scales half0 (+ half1's last chunk so
        # the phase never ends on ACT's serial chain); ACT scales the rest of
        # half1. Stores interleave on the two HWDGE rings with one mid-chunk
        # and the last chunk swapped across rings for byte balance.
        soffs = _offs(STORE_W)
        for c, w in enumerate(STORE_W):
            sl = slice(soffs[c], soffs[c] + w)
            nc.vector.tensor_scalar_mul(
                out=xt0[:, sl], in0=xt0[:, sl], scalar1=y2_sb[:, 0:1]
            )
            if c == LAST:
                nc.vector.tensor_scalar_mul(
                    out=xt1[:, sl], in0=xt1[:, sl], scalar1=y2_sb[:, 1:2]
                )
            else:
                nc.scalar.activation(
                    out=xt1[:, sl], in_=xt1[:, sl], func=AF.Copy, bias=0.0,
                    scale=y2_sb[:, 1:2],
                )
            r0, r1 = (nc.sync, nc.scalar)
            if c in (SWAP_IDX, LAST):
                r0, r1 = r1, r0
            r0.dma_start(out=outr[0, :, sl], in_=xt0[:, sl])
            r1.dma_start(out=outr[1, :, sl], in_=xt1[:, sl])


def build_calayer_bass(trn_type="TRN2"):
    # Bacc (not raw Bass): its compile() pipeline splits multi-wait sync_info
    # into event semaphores — TRN2 instructions encode at most one wait.
    nc = bacc.Bacc(trn_type=trn_type)
    x = nc.dram_tensor("x", [C, S], F32, kind="ExternalInput")
    w1t = nc.dram_tensor("w1t", [C, CS], F32, kind="ExternalInput")
    b1 = nc.dram_tensor("b1", [CS, 1], F32, kind="ExternalInput")
    w2b = nc.dram_tensor("w2b", [CS + 1, C], F32, kind="ExternalInput")
    out = nc.dram_tensor("out", [C, S], F32, kind="ExternalOutput")
    with tile.TileContext(nc) as tc:
        _body(tc, x[:, :], w1t[:, :], b1[:, :], w2b[:, :], out[:, :])
    nc.finalize()  # Bacc.finalize runs compile(): wait-splitting, reg alloc
    return nc


_NC_CACHE = None
RUN_KWARGS = {}      # test harness may inject trace=True etc.
LAST_RESULT = None   # BassKernelResults of the most recent run


def _get_nc():
    global _NC_CACHE
    if _NC_CACHE is None:
        _NC_CACHE = build_calayer_bass()
    return _NC_CACHE


def kernel(x, w1, b1, w2, b2):
    global LAST_RESULT
    x = np.asarray(x, dtype=np.float32)
    xf = np.ascontiguousarray(x.reshape(B, C, S))
    w1t_h = np.ascontiguousarray(np.asarray(w1, dtype=np.float32).T)  # [C, CS]
    w2t_h = np.asarray(w2, dtype=np.float32).T  # [CS, C]
    b1_h = np.ascontiguousarray(np.asarray(b1, dtype=np.float32).reshape(CS, 1))
    b2r = np.asarray(b2, dtype=np.float32).reshape(1, C)
    w2b_h = np.ascontiguousarray(np.concatenate([w2t_h, b2r], axis=0))  # [CS+1, C]

    in_maps = [
        {"x": xf[b], "w1t": w1t_h, "b1": b1_h, "w2b": w2b_h}
        for b in range(B)
    ]
    res = run_bass_kernel_spmd(
        _get_nc(), in_maps, core_ids=list(range(N_CORES)), **RUN_KWARGS
    )
    LAST_RESULT = res
    out = np.stack([res.results[b]["out"] for b in range(B)], axis=0)
    return out.reshape(B, C, H, W)
